# revision 6
# baseline (speedup 1.0000x reference)
"""TGN-style GNN message-passing kernel for 8 Trainium2 NeuronCores.

Sharding: node tables row-sharded 8 ways (25000 rows/core). Each core
  - copies its memory/last_update shard to the output (bulk of the memory
    roofline: 50MB read + 50MB write),
  - runs the GRU memory update for the "last" message of each node it owns
    (operands pre-routed/transposed by the host, float32r matmuls on PE),
  - scatters updated rows into its output shard via indirect DMA,
  - computes 1/8 of the edge-MLP scores (pos+neg batched as M=1024).
Small weights are replicated; the event batch is data-parallel.
"""
import math
import numpy as np

import concourse.bass as bass
import concourse.tile as tile
from concourse import bacc, mybir
from concourse.tile import add_dep_helper
from concourse.bass_utils import run_bass_kernel_spmd

F32 = mybir.dt.float32
F32R = mybir.dt.float32r
I32 = mybir.dt.int32

N = 200000
DM = 500
B = 4096
NCORES = 8
SH = N // NCORES          # 25000 rows per core
BPC = B // NCORES         # 512 events per core
MROW = 1024               # MLP batch rows per core (pos 512 + neg 512)
KABE = 1002               # A.T(500) | ones(1) | B.T(500) | ef(1)
KA1 = 501                 # A.T(500) | ones(1)   (for gh_n matmul)
KX = 2001                 # x.T(2000) | ones(1)
G3 = 1536                 # 3 x 512 psum segments (500 used each)
HALF_PI = float(np.pi / 2)

_CACHE: dict[int, "object"] = {}


def _build(cap: int):
    """Build + compile the 8-core SPMD Bass program for GRU capacity `cap`."""
    import os
    SKIP = set(os.environ.get("KSKIP", "").split(","))
    nc = bacc.Bacc("TRN2", target_bir_lowering=False, debug=False,
                   num_devices=NCORES)
    dp = nc.declare_dram_parameter
    mem_in = dp("mem_in", [SH, DM], F32, isOutput=False)
    lu_in = dp("lu_in", [SH, 1], F32, isOutput=False)
    abet = dp("abet", [KABE, cap], F32R, isOutput=False)
    rs = dp("rs", [KABE, G3], F32R, isOutput=False)
    rnp = dp("rnp", [KA1, DM], F32R, isOutput=False)
    wct = dp("wct", [DM, G3], F32R, isOutput=False)
    twc = dp("twc", [1, DM], F32, isOutput=False)
    tbb = dp("tbb", [DM, 1], F32, isOutput=False)
    dtr = dp("dtr", [1, cap], F32, isOutput=False)
    arows = dp("arows", [cap, DM], F32, isOutput=False)
    uidx = dp("uidx", [cap], I32, isOutput=False)
    tupd = dp("tupd", [cap, 1], F32, isOutput=False)
    xpt = dp("xpt", [KX, MROW], F32R, isOutput=False)
    rw1 = dp("rw1", [KX, 80], F32R, isOutput=False)
    w2bc = dp("w2bc", [128, 800], F32, isOutput=False)
    b2bc = dp("b2bc", [128, 10], F32, isOutput=False)
    w3bc = dp("w3bc", [128, 10], F32, isOutput=False)
    b3bc = dp("b3bc", [128, 1], F32, isOutput=False)

    mem_out = dp("mem_out", [SH, DM], F32, isOutput=True)
    lu_out = dp("lu_out", [SH, 1], F32, isOutput=True)
    spos = dp("spos", [BPC, 1], F32, isOutput=True)
    sneg = dp("sneg", [BPC, 1], F32, isOutput=True)

    n_mt = cap // 128          # GRU M-tiles
    abe_ch = [(c * 128, min((c + 1) * 128, KABE)) for c in range(8)]
    a1_ch = [(0, 128), (128, 256), (256, 384), (384, KA1)]
    x_ch = [(c * 128, min((c + 1) * 128, KX)) for c in range(16)]
    segs = [(0, 500), (512, 1012), (1024, 1524)]  # bank-aligned psum segments

    with tile.TileContext(nc) as tc:
        # ---- bulk shard copy, DRAM->DRAM, chunked for DMA-queue parallelism
        nch = 25
        rows = SH // nch
        if "copy" not in SKIP:
            for i in range(nch):
                nc.sync.dma_start(mem_out[i * rows:(i + 1) * rows, :],
                                  mem_in[i * rows:(i + 1) * rows, :])
            nc.sync.dma_start(lu_out.rearrange("(a b) c -> a (b c)", b=1000),
                              lu_in.rearrange("(a b) c -> a (b c)", b=1000))

        with tc.tile_pool(name="const", bufs=1) as const:
            zb = const.tile([128, 1], F32)
            nc.gpsimd.memset(zb[:], 0.0)

            rs_t = []
            for c, (k0, k1) in enumerate(abe_ch):
                t = const.tile([128, G3], F32R, tag=f"rs{c}")
                nc.gpsimd.dma_start(t[: k1 - k0, :], rs[k0:k1, :])
                rs_t.append(t)
            rnp_t = []
            for c, (k0, k1) in enumerate(a1_ch):
                t = const.tile([128, DM], F32R, tag=f"rnp{c}")
                nc.gpsimd.dma_start(t[: k1 - k0, :], rnp[k0:k1, :])
                rnp_t.append(t)
            wct_t = []
            for c in range(4):
                t = const.tile([125, G3], F32R, tag=f"wct{c}")
                nc.gpsimd.dma_start(t[:], wct[c * 125:(c + 1) * 125, :])
                wct_t.append(t)
            tw_t = const.tile([1, DM], F32)
            nc.gpsimd.dma_start(tw_t[:], twc[:])
            tbb_t = []
            for c in range(4):
                t = const.tile([125, 1], F32, tag=f"tbb{c}")
                nc.gpsimd.dma_start(t[:], tbb[c * 125:(c + 1) * 125, :])
                tbb_t.append(t)
            dtr_t = const.tile([1, cap], F32)
            nc.gpsimd.dma_start(dtr_t[:], dtr[:])
            w2_t = const.tile([128, 800], F32)
            nc.gpsimd.dma_start(w2_t[:], w2bc[:])
            b2_t = const.tile([128, 10], F32)
            nc.gpsimd.dma_start(b2_t[:], b2bc[:])
            w3_t = const.tile([128, 10], F32)
            nc.gpsimd.dma_start(w3_t[:], w3bc[:])
            b3_t = const.tile([128, 1], F32)
            nc.gpsimd.dma_start(b3_t[:], b3bc[:])
            rw1_t = []
            for c, (k0, k1) in enumerate(x_ch):
                t = const.tile([128, 80], F32R, tag=f"rw1{c}")
                nc.gpsimd.dma_start(t[: k1 - k0, :], rw1[k0:k1, :])
                rw1_t.append(t)

            # ---- ENC: enct[c] = sin(tw[c-chunk] x dt + (tb + pi/2)), f32r
            with tc.tile_pool(name="encsb", bufs=1) as encsb:
                enct_t = []
                with tc.tile_pool(name="encps", bufs=2, space="PSUM") as encps:
                    for c in range(4):
                        et = encsb.tile([125, cap], F32R, tag=f"enc{c}")
                        for s in range(cap // 512):
                            pe = encps.tile([125, 512], F32, space="PSUM")
                            nc.tensor.matmul(
                                pe[:], lhsT=tw_t[0:1, c * 125:(c + 1) * 125],
                                rhs=dtr_t[0:1, s * 512:(s + 1) * 512],
                                start=True, stop=True)
                            ef32 = encsb.tile([125, 512], F32, tag="encf32")
                            nc.scalar.activation(
                                ef32[:], pe[:],
                                mybir.ActivationFunctionType.Sin,
                                bias=tbb_t[c][:])
                            nc.vector.tensor_copy(
                                et[:, s * 512:(s + 1) * 512], ef32[:])
                        enct_t.append(et)

                # ---- ABe operand panels resident
                with tc.tile_pool(name="abe", bufs=1) as abep:
                    abe_t = []
                    for c, (k0, k1) in enumerate(abe_ch):
                        t = abep.tile([128, cap], F32R, tag=f"abe{c}")
                        nc.gpsimd.dma_start(t[: k1 - k0, :], abet[k0:k1, :])
                        abe_t.append(t)

                    # ---- GRU M-tile loop
                    with tc.tile_pool(name="sps", bufs=2, space="PSUM") as sps, \
                         tc.tile_pool(name="gps", bufs=2, space="PSUM") as gps, \
                         tc.tile_pool(name="wk", bufs=2) as wk:
                        for t in range(n_mt):
                            ms = slice(t * 128, (t + 1) * 128)
                            sp = sps.tile([128, G3], F32, space="PSUM")
                            for j, (n0, n1) in enumerate(segs):
                                nmm = len(abe_ch) + 4
                                i = 0
                                for c, (k0, k1) in enumerate(abe_ch):
                                    nc.tensor.matmul(
                                        sp[:, n0:n1],
                                        lhsT=abe_t[c][: k1 - k0, ms],
                                        rhs=rs_t[c][: k1 - k0, n0:n1],
                                        start=(i == 0), stop=(i == nmm - 1))
                                    i += 1
                                for c in range(4):
                                    nc.tensor.matmul(
                                        sp[:, n0:n1],
                                        lhsT=enct_t[c][:, ms],
                                        rhs=wct_t[c][:, n0:n1],
                                        start=(i == 0), stop=(i == nmm - 1))
                                    i += 1
                            gp = gps.tile([128, 512], F32, space="PSUM")
                            for c, (k0, k1) in enumerate(a1_ch):
                                nc.tensor.matmul(
                                    gp[:, 0:DM],
                                    lhsT=abe_t[c][: k1 - k0, ms],
                                    rhs=rnp_t[c][: k1 - k0, :],
                                    start=(c == 0), stop=(c == 3))

                            # gates: r_t = tanh(S_r/2), z_t = tanh(S_z/2)
                            r_t = wk.tile([128, DM], F32, tag="rt")
                            nc.scalar.activation(r_t[:], sp[:, 0:500],
                                                 mybir.ActivationFunctionType.Tanh,
                                                 bias=zb[:], scale=0.5)
                            z_t = wk.tile([128, DM], F32, tag="zt")
                            nc.scalar.activation(z_t[:], sp[:, 512:1012],
                                                 mybir.ActivationFunctionType.Tanh,
                                                 bias=zb[:], scale=0.5)
                            # nmix = S_n + (r-1)*gh_n ; r = (r_t+1)/2
                            t1 = wk.tile([128, DM], F32, tag="t1")
                            nc.vector.scalar_tensor_tensor(
                                out=t1[:], in0=r_t[:], scalar=-1.0,
                                in1=gp[:, 0:DM],
                                op0=mybir.AluOpType.add,
                                op1=mybir.AluOpType.mult)
                            nm = wk.tile([128, DM], F32, tag="nm")
                            nc.vector.scalar_tensor_tensor(
                                out=nm[:], in0=t1[:], scalar=0.5,
                                in1=sp[:, 1024:1524],
                                op0=mybir.AluOpType.mult,
                                op1=mybir.AluOpType.add)
                            n_t = wk.tile([128, DM], F32, tag="nt")
                            nc.scalar.activation(n_t[:], nm[:],
                                                 mybir.ActivationFunctionType.Tanh,
                                                 bias=zb[:])
                            # h_new = n + z*(h-n) ; z = (z_t+1)/2
                            a_t = wk.tile([128, DM], F32, tag="at")
                            nc.gpsimd.dma_start(a_t[:], arows[ms, :])
                            d_t = wk.tile([128, DM], F32, tag="dt")
                            nc.vector.tensor_sub(d_t[:], a_t[:], n_t[:])
                            e_t = wk.tile([128, DM], F32, tag="et")
                            nc.vector.scalar_tensor_tensor(
                                out=e_t[:], in0=z_t[:], scalar=1.0, in1=d_t[:],
                                op0=mybir.AluOpType.add,
                                op1=mybir.AluOpType.mult)
                            hn = wk.tile([128, DM], F32, tag="hn")
                            nc.vector.scalar_tensor_tensor(
                                out=hn[:], in0=e_t[:], scalar=0.5, in1=n_t[:],
                                op0=mybir.AluOpType.mult,
                                op1=mybir.AluOpType.add)

                            ix = wk.tile([128, 1], I32, tag="ix")
                            nc.gpsimd.dma_start(ix[:], uidx[ms, None])
                            if "scatter" not in SKIP:
                                nc.gpsimd.indirect_dma_start(
                                    out=mem_out[:],
                                    out_offset=bass.IndirectOffsetOnAxis(
                                        ap=ix[:, :1], axis=0),
                                    in_=hn[:], in_offset=None,
                                    bounds_check=SH - 1, oob_is_err=False)
                            else:
                                nc.sync.dma_start(
                                    mem_out[0:128, :], hn[:])
                            tu = wk.tile([128, 1], F32, tag="tu")
                            nc.gpsimd.dma_start(tu[:], tupd[ms, :])
                            if "scatter" not in SKIP:
                                nc.gpsimd.indirect_dma_start(
                                    out=lu_out[:],
                                    out_offset=bass.IndirectOffsetOnAxis(
                                        ap=ix[:, :1], axis=0),
                                    in_=tu[:], in_offset=None,
                                    bounds_check=SH - 1, oob_is_err=False)

            # ---- MLP edge scores (pos rows 0..511 = tiles 0-3, neg 4-7)
            with tc.tile_pool(name="xp", bufs=1) as xpp, \
                 tc.tile_pool(name="mps", bufs=2, space="PSUM") as mps, \
                 tc.tile_pool(name="mwk", bufs=2) as mwk:
                xp_t = []
                for c, (k0, k1) in enumerate(x_ch):
                    t = xpp.tile([128, MROW], F32R, tag=f"xp{c}")
                    nc.gpsimd.dma_start(t[: k1 - k0, :], xpt[k0:k1, :])
                    xp_t.append(t)
                for t in range(MROW // 128):
                    ms = slice(t * 128, (t + 1) * 128)
                    pp = mps.tile([128, 80], F32, space="PSUM")
                    for c, (k0, k1) in enumerate(x_ch):
                        nc.tensor.matmul(pp[:], lhsT=xp_t[c][: k1 - k0, ms],
                                         rhs=rw1_t[c][: k1 - k0, :],
                                         start=(c == 0), stop=(c == 15))
                    h1 = mwk.tile([128, 80], F32, tag="h1")
                    nc.scalar.activation(h1[:], pp[:],
                                         mybir.ActivationFunctionType.Relu,
                                         bias=zb[:])
                    h2 = mwk.tile([128, 10], F32, tag="h2")
                    scr = mwk.tile([128, 80], F32, tag="scr")
                    for j in range(10):
                        nc.vector.tensor_mul(scr[:], h1[:],
                                             w2_t[:, j * 80:(j + 1) * 80])
                        nc.vector.reduce_sum(h2[:, j:j + 1], scr[:],
                                             axis=mybir.AxisListType.X)
                    nc.vector.tensor_add(h2[:], h2[:], b2_t[:])
                    h2r = mwk.tile([128, 10], F32, tag="h2r")
                    nc.scalar.activation(h2r[:], h2[:],
                                         mybir.ActivationFunctionType.Relu,
                                         bias=zb[:])
                    scr2 = mwk.tile([128, 10], F32, tag="scr2")
                    s_t = mwk.tile([128, 1], F32, tag="st")
                    nc.vector.tensor_mul(scr2[:], h2r[:], w3_t[:])
                    nc.vector.reduce_sum(s_t[:, 0:1], scr2[:],
                                         axis=mybir.AxisListType.X)
                    nc.vector.tensor_add(s_t[:], s_t[:], b3_t[:])
                    if t < 4:
                        nc.sync.dma_start(spos[t * 128:(t + 1) * 128, :], s_t[:])
                    else:
                        nc.sync.dma_start(sneg[(t - 4) * 128:(t - 3) * 128, :],
                                          s_t[:])

    nc.compile()
    return nc


def _get_program(cap: int):
    if cap not in _CACHE:
        _CACHE[cap] = _build(cap)
    return _CACHE[cap]


def _route_and_stage(inputs, cap=None):
    memory = np.ascontiguousarray(np.asarray(inputs["memory"], np.float32))
    last_update = np.asarray(inputs["last_update"], np.float32)
    community = np.asarray(inputs["community_emb"], np.float32)
    eft = np.asarray(inputs["edge_feat_table"], np.float32)
    et = np.asarray(inputs["edge_times"], np.float32)
    time_w = np.asarray(inputs["time_w"], np.float32)
    time_b = np.asarray(inputs["time_b"], np.float32)
    W_ih = np.asarray(inputs["W_ih"], np.float32)
    W_hh = np.asarray(inputs["W_hh"], np.float32)
    b_ih = np.asarray(inputs["b_ih"], np.float32)
    b_hh = np.asarray(inputs["b_hh"], np.float32)
    W1 = np.asarray(inputs["W1"], np.float32)
    b1 = np.asarray(inputs["b1"], np.float32)
    W2 = np.asarray(inputs["W2"], np.float32)
    b2 = np.asarray(inputs["b2"], np.float32)
    W3 = np.asarray(inputs["W3"], np.float32)
    b3 = np.asarray(inputs["b3"], np.float32)
    src = np.asarray(inputs["src"], np.int64)
    dst = np.asarray(inputs["dst"], np.int64)
    neg = np.asarray(inputs["neg"], np.int64)
    eidx = np.asarray(inputs["edge_idxs"], np.int64)

    # ---- 'last' aggregation routing
    nodes = np.concatenate([src, dst])
    second = np.concatenate([dst, src])
    times2 = np.concatenate([et, et])
    ef2 = np.concatenate([eft[eidx, 0], eft[eidx, 0]])
    pos = np.arange(2 * B, dtype=np.int64)
    last_pos = np.full(N, -1, np.int64)
    np.maximum.at(last_pos, nodes, pos)
    m_idx = np.nonzero(last_pos[nodes] == pos)[0]
    node_m = nodes[m_idx]
    core_m = node_m // SH
    per_core_m = [m_idx[core_m == k] for k in range(NCORES)]
    need = max(len(s) for s in per_core_m)
    if cap is None:
        cap = max(1536, ((need + 127) // 128) * 128)
    assert need <= cap

    # ---- shared weight panels
    rs_np = np.zeros((KABE, G3), np.float32)
    wct_np = np.zeros((DM, G3), np.float32)
    rnp_np = np.zeros((KA1, DM), np.float32)
    bsum = b_ih + b_hh
    for j, (n0, _) in enumerate([(0, 0), (512, 0), (1024, 0)]):
        cols = slice(j * DM, (j + 1) * DM)
        rs_np[0:DM, n0:n0 + DM] = (W_ih[cols, 0:DM] + W_hh[cols, :]).T
        rs_np[DM, n0:n0 + DM] = bsum[cols]
        rs_np[DM + 1:2 * DM + 1, n0:n0 + DM] = W_ih[cols, DM:2 * DM].T
        rs_np[2 * DM + 1, n0:n0 + DM] = W_ih[cols, 2 * DM]
        wct_np[:, n0:n0 + DM] = W_ih[cols, 2 * DM + 1:].T
    rnp_np[0:DM, :] = W_hh[2 * DM:3 * DM, :].T
    rnp_np[DM, :] = b_hh[2 * DM:3 * DM]

    rw1_np = np.zeros((KX, 80), np.float32)
    rw1_np[0:2000, :] = W1
    rw1_np[2000, :] = b1
    w2bc = np.broadcast_to(W2.T.reshape(1, 800), (128, 800)).copy()
    b2bc = np.broadcast_to(b2.reshape(1, 10), (128, 10)).copy()
    w3bc = np.broadcast_to(W3[:, 0].reshape(1, 10), (128, 10)).copy()
    b3bc = np.full((128, 1), b3[0], np.float32)
    twc = time_w.reshape(1, DM)
    tbb = (time_b + HALF_PI).reshape(DM, 1)

    in_maps = []
    for k in range(NCORES):
        sel = per_core_m[k]
        mk = len(sel)
        first = node_m[core_m == k]
        sec = second[sel]
        abet = np.zeros((KABE, cap), np.float32)
        abet[0:DM, :mk] = memory[first].T
        abet[DM, :mk] = 1.0
        abet[DM + 1:2 * DM + 1, :mk] = memory[sec].T
        abet[2 * DM + 1, :mk] = ef2[sel]
        dtr = np.zeros((1, cap), np.float32)
        dtr[0, :mk] = times2[sel] - last_update[first]
        arows = np.zeros((cap, DM), np.float32)
        arows[:mk] = memory[first]
        uidx = np.full(cap, SH, np.int32)
        uidx[:mk] = (first - k * SH).astype(np.int32)
        tupd = np.zeros((cap, 1), np.float32)
        tupd[:mk, 0] = times2[sel]

        ev = slice(k * BPC, (k + 1) * BPC)
        xpt = np.zeros((KX, MROW), np.float32)
        ms_, cs_ = memory[src[ev]].T, community[src[ev]].T
        xpt[0:DM, 0:BPC] = ms_
        xpt[DM:2 * DM, 0:BPC] = cs_
        xpt[2 * DM:3 * DM, 0:BPC] = memory[dst[ev]].T
        xpt[3 * DM:4 * DM, 0:BPC] = community[dst[ev]].T
        xpt[0:DM, BPC:] = ms_
        xpt[DM:2 * DM, BPC:] = cs_
        xpt[2 * DM:3 * DM, BPC:] = memory[neg[ev]].T
        xpt[3 * DM:4 * DM, BPC:] = community[neg[ev]].T
        xpt[2000, :] = 1.0

        in_maps.append({
            "mem_in": memory[k * SH:(k + 1) * SH],
            "lu_in": last_update[k * SH:(k + 1) * SH].reshape(SH, 1),
            "abet": abet, "rs": rs_np, "rnp": rnp_np, "wct": wct_np,
            "twc": twc, "tbb": tbb, "dtr": dtr, "arows": arows,
            "uidx": uidx, "tupd": tupd, "xpt": xpt, "rw1": rw1_np,
            "w2bc": w2bc, "b2bc": b2bc, "w3bc": w3bc, "b3bc": b3bc,
        })
    return in_maps, cap


def bench_exec_ns(inputs, iters=5):
    """Time the on-device execution (inputs device-resident, donated zero
    outputs pre-staged) and return the best per-iteration time in ns."""
    import time as _time
    import jax
    from jax.experimental.shard_map import shard_map
    from jax.sharding import Mesh, NamedSharding, PartitionSpec
    from concourse import bass2jax, mybir as _mb

    in_maps, cap = _route_and_stage(inputs)
    nc = _get_program(cap)
    bass2jax.install_neuronx_cc_hook()

    in_names, out_names, out_avals, zero_outs = [], [], [], []
    pname = nc.partition_id_tensor.name if nc.partition_id_tensor else None
    for alloc in nc.m.functions[0].allocations:
        if not isinstance(alloc, _mb.MemoryLocationSet):
            continue
        name = alloc.memorylocations[0].name
        if alloc.kind == "ExternalInput":
            if name != pname:
                in_names.append(name)
        elif alloc.kind == "ExternalOutput":
            shape = tuple(alloc.tensor_shape)
            dtype = _mb.dt.np(alloc.dtype)
            out_names.append(name)
            out_avals.append(jax.core.ShapedArray(shape, dtype))
            zero_outs.append(np.zeros(shape, dtype))
    n_params = len(in_names)
    all_in_names = list(in_names) + list(out_names)
    if pname is not None:
        all_in_names.append(pname)
    donate = tuple(range(n_params, n_params + len(out_names)))

    def _body(*args):
        operands = list(args)
        if pname is not None:
            operands.append(bass2jax.partition_id_tensor())
        return tuple(bass2jax._bass_exec_p.bind(
            *operands, out_avals=tuple(out_avals),
            in_names=tuple(all_in_names), out_names=tuple(out_names),
            lowering_input_output_aliases=(), sim_require_finite=True,
            sim_require_nnan=True, nc=nc))

    devices = jax.devices()[:NCORES]
    mesh = Mesh(np.asarray(devices), ("core",))
    nin = n_params + len(out_names)
    fn = jax.jit(shard_map(_body, mesh=mesh,
                           in_specs=(PartitionSpec("core"),) * nin,
                           out_specs=(PartitionSpec("core"),) * len(out_names),
                           check_rep=False),
                 donate_argnums=donate, keep_unused=True)
    shard = NamedSharding(mesh, PartitionSpec("core"))
    concat_in = [
        jax.device_put(
            np.concatenate([np.asarray(in_maps[c][nm]) for c in range(NCORES)],
                           axis=0), shard)
        for nm in in_names
    ]
    zero_sets = []
    for _ in range(iters + 1):
        zero_sets.append([
            jax.device_put(np.zeros((NCORES * z.shape[0], *z.shape[1:]),
                                    z.dtype), shard)
            for z in zero_outs
        ])
    # warmup (compile)
    jax.block_until_ready(fn(*concat_in, *zero_sets[0]))
    best = float("inf")
    for i in range(iters):
        t0 = _time.perf_counter()
        jax.block_until_ready(fn(*concat_in, *zero_sets[i + 1]))
        best = min(best, _time.perf_counter() - t0)
    return best * 1e9


def kernel(**inputs):
    in_maps, cap = _route_and_stage(inputs)
    nc = _get_program(cap)
    res = run_bass_kernel_spmd(nc, in_maps, core_ids=list(range(NCORES)))
    r = res.results
    score_pos = np.concatenate([r[k]["spos"][:, 0] for k in range(NCORES)])
    score_neg = np.concatenate([r[k]["sneg"][:, 0] for k in range(NCORES)])
    new_memory = np.concatenate([r[k]["mem_out"] for k in range(NCORES)], axis=0)
    new_last_update = np.concatenate(
        [r[k]["lu_out"][:, 0] for k in range(NCORES)])
    return score_pos, score_neg, new_memory, new_last_update
